# revision 33
# baseline (speedup 1.0000x reference)
"""BatchHardTripletLoss on 8 Trainium2 NeuronCores — v12 (host-normalized
fp8, per-core label-localizing permutation, wavefront pipeline).

Math (rows sorted by label; host pre-normalizes):
  en_j = Q(8 * e_j / ||e_j||)    (fp8 e4m3, host)
  ps   = en_blk @ en             (Gram block = 64*s_ij, fp8 DoubleRow)
  p16  = f16(ps); region slots get p16 + (-192*eq)  (gpsimd add, in place)
  M    = max_j p16 (quad folds into slot s0);  m = min over region slice
  loss_row = relu(M - m - 172.8); host: mean(loss)/64

Why 192: |64*s| <= 64+eps, so same-label tw <= -128+eps < -64-eps <= any
negative — max over all = 64*max_neg s, min over slice = 64*min_pos-192,
M - m - (192 - 0.3*64) = 64*(max_neg - min_pos + 0.3). Unique-label rows:
M - m < 172 -> relu 0, matching the reference's empty-positive convention.

Per-core column permutation (the key trick): core c owns row-tiles
g = 8m + c; tile m's same-label columns (<= ~170, since its 8 tiles are
1024 rows apart and label runs are short) are gathered into region
R_m = [256m, 256m+256) of that core's private column order, padded with
arbitrary other columns. Labels crossing a 128-row tile boundary are
assigned to the earlier tile's region, so tile m's eq/min slice is
[256(m-1), 256m+512) — 512 wide, fixed per m, inside quad 0. This
replaces v5's 1408-wide windows: the min reduce is one 512-wide 1x op,
the gpsimd eq add is 512 wide, and eqm is 0.5MB fp8.

Engine split per (q,m) iteration (steady ~2us, scalar/tensor co-paced):
  tensor: 8x (LDW + fp8-DR matmul) -> ps [128,2048] f32 PSUM  (~1.7us)
  scalar: ONE copy PSUM f32 -> SBUF f16 slot (1966ns; sets the pace)
  gpsimd: 512-wide eq add on the q0 slot (add/mult are all Q7 has)
  vector: lagged 1-2 iterations: 2048-wide TT-max fold of the slot into
          s0 (2x_1p f16), 512-wide min reduce, per-m finale folds+reduce
Iteration order is a wavefront over (position, m) anti-diagonals so each
m's quads are ~4 iterations apart: fold work streams uniformly and the
only DVE tail is the last m's fold+finale. PE warmup matmuls on garbage
SBUF absorb the p-state ramp during the DMA fill.

DMA: each dma_start queues on the ISSUING engine's DGE queue. An engine
blocks after ~4 outstanding templates, and the sync/SP queue is ~5-30GB/s
(its engine is busy with semaphores), so: scalar queue gets ETq0+ETq3,
gpsimd queue gets BlkT+ETq1+eqm+ETq2 (few, large templates), sync gets
only the final 512B out. PSUM-direct DVE folds were tried three times and
always regress: with bufs=2, any DVE consumer of a ps tile couples the
tensor engine to the DVE queue through buffer recycling.

HW pitfalls baked in: tensor_tensor_reduce crashes the exec unit — not
used. GPSIMD cannot touch PSUM; walrus rejects gpsimd min/max/free-axis
reduce (only Add/Multiply have Q7 impls). pool_max fails ISA checks.
Matmul moving free dim is capped at 512 (s3d3_mm_num_elements).
walrus --enable-ldw-opt=true crashes codegen. Tile dependencies follow
emission order. ~10us of postamble (semaphore resets + end barrier) is
framework-fixed and included in HW exec time.
"""

import numpy as np
from contextlib import ExitStack

N, D = 8192, 512
NCORES = 8
M_TILES = 8
K_TILES = D // 128   # 4
NQ = 4
QW = 2048
WMAX = 512
GSCALE = 8.0         # host fp8 quantize scale; Gram = 64*s
EQV = 192.0          # same-label offset in 64*s units (fp8-exact)
MARGIN_C = EQV - 0.3 * 64.0    # 172.8
POSF = 30000.0


def _window(m):
    # per-core permuted layout: tile m's same-label columns live in region
    # R_m = [256m, 256m+256); the min/eq slice also covers R_{m-1} because
    # labels crossing the 128-row tile boundary are assigned to the earlier
    # tile's region. All windows live in quad 0.
    lo = max(0, 256 * m - 256)
    hi = 256 * m + 256
    return lo, hi


def _pieces(q, m):
    """Split quad q's [qlo,qhi) columns into (lo, hi, is_window) spans."""
    wlo, whi = _window(m)
    qlo, qhi = q * QW, (q + 1) * QW
    a, b = max(qlo, wlo), min(qhi, whi)
    out = []
    if a >= b:
        out.append((qlo, qhi, False))
    else:
        if qlo < a:
            out.append((qlo, a, False))
        out.append((a, b, True))
        if b < qhi:
            out.append((b, qhi, False))
    return out


def _rot(m):
    """Per-m quad processing order, starting at the window's first quad so
    min/eq work lands in the first two of m's iterations, not the last."""
    wq = _window(m)[0] // QW
    return [(wq + i) % NQ for i in range(NQ)]


def _slot(pos):
    return 0 if pos == 0 else (2 if pos % 2 == 1 else 1)


class TileCtx:
    def __init__(self, nc, tile_mod):
        self.nc = nc
        self.tile_mod = tile_mod

    def __enter__(self):
        self.ctx = ExitStack()
        self.ctx.__enter__()
        self.tc = self.tile_mod.TileContext(self.nc)
        self.tc.__enter__()
        return self.tc, self.ctx

    def __exit__(self, *exc):
        self.ctx.__exit__(*exc)
        return self.tc.__exit__(*exc)


def _build_program():
    import concourse.bass as bass
    import concourse.bacc as bacc
    import concourse.tile as tile
    from concourse import mybir

    f16 = mybir.dt.float16
    f32 = mybir.dt.float32
    f8 = mybir.dt.float8e4
    Alu = mybir.AluOpType
    Act = mybir.ActivationFunctionType
    Ax = mybir.AxisListType
    DR = mybir.MatmulPerfMode.DoubleRow

    nc = bacc.Bacc("TRN2", target_bir_lowering=False, debug=False,
                   num_devices=NCORES)

    embT = nc.dram_tensor("embT", [D, N], f8, kind="ExternalInput").ap()
    blkT = nc.dram_tensor("blkT", [128, K_TILES * 1024], f8,
                          kind="ExternalInput").ap()
    eqm = nc.dram_tensor("eqm", [128, M_TILES * WMAX], f8,
                         kind="ExternalInput").ap()
    out = nc.dram_tensor("out", [128, 1], f32, kind="ExternalOutput").ap()

    with TileCtx(nc, tile) as (tc, ctx):
        persist = ctx.enter_context(tc.tile_pool(name="persist", bufs=1))
        psum = ctx.enter_context(tc.tile_pool(name="ps", bufs=2, space="PSUM"))

        ETq = [persist.tile([128, K_TILES, QW], f8, tag=f"etq{g}",
                            name=f"etq{g}") for g in range(NQ)]
        BlkT = persist.tile([128, K_TILES, 1024], f8, tag="blkt")
        EQM = persist.tile([128, M_TILES * WMAX], f8, tag="eqm")
        P16 = persist.tile([128, 3 * M_TILES, QW], f16, tag="p16")
        minp = persist.tile([128, 2, M_TILES], f32, tag="minp")
        maxF = persist.tile([128, M_TILES], f32, tag="maxF")
        minF = persist.tile([128, M_TILES], f32, tag="minF")
        diffs = persist.tile([128, M_TILES], f32, tag="diffs")
        relu_d = persist.tile([128, M_TILES], f32, tag="relud")
        row_loss = persist.tile([128, 1], f32, tag="rowloss")
        negm = persist.tile([128, 1], f32, tag="negm")

        nc.vector.memset(minp[:], POSF)
        nc.vector.memset(negm[:], -MARGIN_C)

        # PE p-state warmup: ~3us of dummy DR matmuls on garbage SBUF (ETq3
        # is DMA'd last, so reading it now costs nothing); the PE reaches
        # full clock before the first real matmul group.
        wps = psum.tile([128, QW], f32, tag="ps")
        for i in range(16):
            nc.tensor.matmul(wps[:, (i % 4) * 512:(i % 4) * 512 + 512],
                             lhsT=ETq[3][:, 0:2, 0:128],
                             rhs=ETq[3][:, 0:2, 0:512],
                             start=True, stop=True, perf_mode=DR)
        nc.vector.tensor_reduce(out=negm[:], in_=wps[:, 0:8], axis=Ax.X,
                                op=Alu.max)
        nc.vector.memset(negm[:], -MARGIN_C)

        # ---------- loads: 3 parallel DGE queues (sync/scalar/gpsimd) ----
        def dma_etq(eng, g, k):
            eng.dma_start(
                out=ETq[g][:, k, :],
                in_=bass.AP(embT.tensor,
                            embT.offset + k * 128 * N + g * QW,
                            [[N, 128], [1, QW]]))

        # eqm is tiny (0.5MB) — park it all on the slow sync queue, which
        # must stay clear for the final out DMA; ETq/BlkT go on the fast
        # scalar/gpsimd DGE queues as few big dispatches (a queue ring holds
        # ~512 descriptors; more dispatches than that block the engine).
        def dma_quad(eng, g):
            eng.dma_start(
                out=ETq[g][:],
                in_=bass.AP(embT.tensor, embT.offset + g * QW,
                            [[N, 128], [N * 128, K_TILES], [1, QW]]))

        nc.gpsimd.dma_start(out=BlkT[:], in_=blkT)
        dma_quad(nc.scalar, 0)
        dma_quad(nc.gpsimd, 1)
        nc.gpsimd.dma_start(
            out=EQM[:],
            in_=bass.AP(eqm.tensor, eqm.offset,
                        [[M_TILES * WMAX, 128], [1, M_TILES * WMAX]]))
        dma_quad(nc.gpsimd, 2)
        dma_quad(nc.scalar, 3)

        # ---------- mining: m outer, q inner, DVE lagged one iteration ----
        wcnt = [0] * M_TILES

        def lagged(m, q, pos):
            """DVE work for (m,q), emitted one iteration later."""
            sl = _slot(pos)
            slot = P16[:, 3 * m + sl, :]
            qlo = q * QW
            wlo, whi = _window(m)
            for (lo, hi, isw) in _pieces(q, m):
                if not isw:
                    continue
                wc = wcnt[m]
                wcnt[m] += 1
                nc.vector.tensor_reduce(
                    out=minp[:, wc, m:m + 1],
                    in_=slot[:, lo - qlo:hi - qlo],
                    axis=Ax.X, op=Alu.min)
            if pos > 0:
                s0 = P16[:, 3 * m, :]
                if pos == NQ - 1:
                    # s0 was self-folded to [0:1024) during the iteration;
                    # fold the last slot's halves into that
                    nc.vector.tensor_tensor(out=s0[:, 0:1024],
                                            in0=s0[:, 0:1024],
                                            in1=slot[:, 0:1024], op=Alu.max)
                    nc.vector.tensor_tensor(out=s0[:, 0:1024],
                                            in0=s0[:, 0:1024],
                                            in1=slot[:, 1024:2048],
                                            op=Alu.max)
                else:
                    nc.vector.tensor_tensor(out=s0[:], in0=s0[:],
                                            in1=slot[:], op=Alu.max)

        def finale(m):
            s0 = P16[:, 3 * m, :]
            nc.vector.tensor_tensor(out=s0[:, 0:512], in0=s0[:, 0:512],
                                    in1=s0[:, 512:1024], op=Alu.max)
            nc.vector.tensor_reduce(
                out=maxF[:, m:m + 1], in_=s0[:, 0:512],
                axis=Ax.X, op=Alu.max)

        deferred = []

        def flush(now):
            keep = []
            for due, fn in deferred:
                if due <= now:
                    fn()
                else:
                    keep.append((due, fn))
            deferred[:] = keep

        order = [(_rot(w - p)[p], w - p, p) for w in range(NQ + M_TILES - 1)
                 for p in range(NQ) if 0 <= w - p < M_TILES]
        if True:
            for it, (q, m, pos) in enumerate(order):
                qlo = q * QW
                wlo, whi = _window(m)
                ps = psum.tile([128, QW], f32, tag="ps")
                for j in range(2):
                    lhsT = BlkT[:, 2 * j:2 * j + 2, m * 128:(m + 1) * 128]
                    rhs_t = ETq[q][:, 2 * j:2 * j + 2, :]
                    for c in range(4):
                        nc.tensor.matmul(
                            ps[:, c * 512:(c + 1) * 512],
                            lhsT=lhsT,
                            rhs=rhs_t[:, :, c * 512:(c + 1) * 512],
                            start=(j == 0), stop=(j == 1), perf_mode=DR)

                slot = P16[:, 3 * m + _slot(pos), :]
                nc.scalar.copy(slot, ps[:])
                for (lo, hi, isw) in _pieces(q, m):
                    if not isw:
                        continue
                    twb = m * WMAX
                    nc.gpsimd.tensor_tensor(
                        out=slot[:, lo - qlo:hi - qlo],
                        in0=slot[:, lo - qlo:hi - qlo],
                        in1=EQM[:, twb + lo - wlo:twb + hi - wlo],
                        op=Alu.add)
                if pos == NQ - 1:
                    # pre-shrink s0 while this iteration's copy streams, so
                    # the post-copy chain for the last m is shorter
                    s0p = P16[:, 3 * m, :]
                    nc.vector.tensor_tensor(out=s0p[:, 0:1024],
                                            in0=s0p[:, 0:1024],
                                            in1=s0p[:, 1024:2048],
                                            op=Alu.max)
                flush(it)
                has_win = any(w[2] for w in _pieces(q, m))
                due = it + 2 if has_win else it + 1
                deferred.append((due, (lambda mm=m, qq=q, pp=pos:
                                       lagged(mm, qq, pp))))
                if pos == NQ - 1:
                    deferred.append((it + 2, (lambda mm=m: finale(mm))))
        flush(10 ** 9)

        # ---------- finale ----------
        nc.vector.tensor_tensor(out=minF[:], in0=minp[:, 0, :],
                                in1=minp[:, 1, :], op=Alu.min)
        nc.vector.tensor_tensor(out=diffs[:], in0=maxF[:], in1=minF[:],
                                op=Alu.subtract)
        nc.scalar.activation(relu_d[:], diffs[:], Act.Relu, bias=negm[:],
                             accum_out=row_loss[:])
        nc.sync.dma_start(out=out, in_=row_loss[:])

    nc.compile()
    return nc


def _prep_inputs(embeddings, labels):
    import ml_dtypes
    E = np.ascontiguousarray(np.asarray(embeddings, dtype=np.float32))
    lab = np.asarray(labels).reshape(-1)
    assert E.shape == (N, D)

    order = np.argsort(lab, kind="stable")
    E_s = E[order]
    lab_s = lab[order].astype(np.int64)
    assert np.bincount(lab_s).max() <= 129, "label multiplicity > 129"

    En = E_s * (GSCALE / np.linalg.norm(E_s, axis=1, keepdims=True))
    E8 = En.astype(ml_dtypes.float8_e4m3)

    # label -> column range in the sorted order
    starts = np.searchsorted(lab_s, np.arange(lab_s.max() + 2))
    tiles8 = E8.reshape(64, 128, D)
    labt = lab_s.reshape(64, 128)
    in_maps = []
    for c in range(NCORES):
        gsel = [8 * m + c for m in range(M_TILES)]
        blk8 = np.ascontiguousarray(tiles8[gsel].reshape(128 * M_TILES, D))
        blkT_c = np.ascontiguousarray(
            blk8.reshape(1024, K_TILES, 128).transpose(2, 1, 0)
            .reshape(128, K_TILES * 1024))
        lab_blk = labt[gsel].reshape(M_TILES, 128)

        # per-core column permutation: tile m's same-label columns (its
        # "union") go to region [256m, 256m+256), padded with leftovers
        used = np.zeros(N, bool)
        unions = []
        for m in range(M_TILES):
            labs = np.unique(lab_blk[m])
            cols = np.concatenate([np.arange(starts[l], starts[l + 1])
                                   for l in labs])
            assert len(cols) <= 256, f"union too wide: {len(cols)}"
            unions.append(cols)
            used[cols] = True
        leftovers = np.nonzero(~used)[0]
        perm = np.empty(N, np.int64)
        li = 0
        for m in range(M_TILES):
            u = unions[m]
            npad = 256 - len(u)
            perm[256 * m:256 * m + len(u)] = u
            perm[256 * m + len(u):256 * (m + 1)] = leftovers[li:li + npad]
            li += npad
        perm[256 * M_TILES:] = leftovers[li:]
        lab_p = lab_s[perm]
        embT_c = np.ascontiguousarray(E8[perm].T)

        # partition-major [128, M*WMAX]: one contiguous 4KB DMA line per
        # partition (128 descriptors instead of 1024 — completion-semaphore
        # processing of tiny descriptors was gating the end barrier ~10us
        # after the last compute)
        eqm_c = np.zeros((128, M_TILES * WMAX), np.float32)
        for m in range(M_TILES):
            wlo, whi = _window(m)
            eqm_c[:, m * WMAX:m * WMAX + whi - wlo] = -EQV * (
                lab_p[None, wlo:whi] == lab_blk[m][:, None])
        eqm_c = eqm_c.astype(ml_dtypes.float8_e4m3)
        in_maps.append({
            "embT": embT_c,
            "blkT": blkT_c,
            "eqm": eqm_c,
        })
    return in_maps


def kernel(embeddings, labels):
    from concourse.bass_utils import run_bass_kernel_spmd

    in_maps = _prep_inputs(embeddings, labels)
    nc = _build_program()
    res = run_bass_kernel_spmd(nc, in_maps, core_ids=list(range(NCORES)))
    global LAST_RESULTS
    LAST_RESULTS = res
    total = sum(float(r["out"].sum()) for r in res.results)
    return np.float32(total / (64.0 * N))


LAST_RESULTS = None


# revision 34
# speedup vs baseline: 1.0313x; 1.0313x over previous
"""BatchHardTripletLoss on 8 Trainium2 NeuronCores — v12 (host-normalized
fp8, per-core label-localizing permutation, wavefront pipeline).

Math (rows sorted by label; host pre-normalizes):
  en_j = Q(8 * e_j / ||e_j||)    (fp8 e4m3, host)
  ps   = en_blk @ en             (Gram block = 64*s_ij, fp8 DoubleRow)
  p16  = f16(ps); region slots get p16 + (-192*eq)  (gpsimd add, in place)
  M    = max_j p16 (quad folds into slot s0);  m = min over region slice
  loss_row = relu(M - m - 172.8); host: mean(loss)/64

Why 192: |64*s| <= 64+eps, so same-label tw <= -128+eps < -64-eps <= any
negative — max over all = 64*max_neg s, min over slice = 64*min_pos-192,
M - m - (192 - 0.3*64) = 64*(max_neg - min_pos + 0.3). Unique-label rows:
M - m < 172 -> relu 0, matching the reference's empty-positive convention.

Per-core column permutation (the key trick): core c owns row-tiles
g = 8m + c; tile m's same-label columns (<= ~170, since its 8 tiles are
1024 rows apart and label runs are short) are gathered into region
R_m = [256m, 256m+256) of that core's private column order, padded with
arbitrary other columns. Labels crossing a 128-row tile boundary are
assigned to the earlier tile's region, so tile m's eq/min slice is
[256(m-1), 256m+512) — 512 wide, fixed per m, inside quad 0. This
replaces v5's 1408-wide windows: the min reduce is one 512-wide 1x op,
the gpsimd eq add is 512 wide, and eqm is 0.5MB fp8.

Engine split per (q,m) iteration (steady ~2us, scalar/tensor co-paced):
  tensor: 8x (LDW + fp8-DR matmul) -> ps [128,2048] f32 PSUM  (~1.7us)
  scalar: ONE copy PSUM f32 -> SBUF f16 slot (1966ns; sets the pace)
  gpsimd: 512-wide eq add on the q0 slot (add/mult are all Q7 has)
  vector: lagged 1-2 iterations: 2048-wide TT-max fold of the slot into
          s0 (2x_1p f16), 512-wide min reduce, per-m finale folds+reduce
Iteration order is a wavefront over (position, m) anti-diagonals so each
m's quads are ~4 iterations apart: fold work streams uniformly and the
only DVE tail is the last m's fold+finale. PE warmup matmuls on garbage
SBUF absorb the p-state ramp during the DMA fill.

DMA: each dma_start queues on the ISSUING engine's DGE queue. An engine
blocks after ~4 outstanding templates, and the sync/SP queue is ~5-30GB/s
(its engine is busy with semaphores), so: scalar queue gets ETq0+ETq3,
gpsimd queue gets BlkT+ETq1+eqm+ETq2 (few, large templates), sync gets
only the final 512B out. PSUM-direct DVE folds were tried three times and
always regress: with bufs=2, any DVE consumer of a ps tile couples the
tensor engine to the DVE queue through buffer recycling.

HW pitfalls baked in: tensor_tensor_reduce crashes the exec unit — not
used. GPSIMD cannot touch PSUM; walrus rejects gpsimd min/max/free-axis
reduce (only Add/Multiply have Q7 impls). pool_max fails ISA checks.
Matmul moving free dim is capped at 512 (s3d3_mm_num_elements).
walrus --enable-ldw-opt=true crashes codegen. Tile dependencies follow
emission order. ~10us of postamble (semaphore resets + end barrier) is
framework-fixed and included in HW exec time.
"""

import numpy as np
from contextlib import ExitStack

N, D = 8192, 512
NCORES = 8
M_TILES = 8
K_TILES = D // 128   # 4
NQ = 4
QW = 2048
WMAX = 512
GSCALE = 8.0         # host fp8 quantize scale; Gram = 64*s
EQV = 192.0          # same-label offset in 64*s units (fp8-exact)
MARGIN_C = EQV - 0.3 * 64.0    # 172.8
POSF = 30000.0


def _window(m):
    # per-core permuted layout: tile m's same-label columns live in region
    # R_m = [256m, 256m+256); the min/eq slice also covers R_{m-1} because
    # labels crossing the 128-row tile boundary are assigned to the earlier
    # tile's region. All windows live in quad 0.
    lo = max(0, 256 * m - 256)
    hi = 256 * m + 256
    return lo, hi


def _pieces(q, m):
    """Split quad q's [qlo,qhi) columns into (lo, hi, is_window) spans."""
    wlo, whi = _window(m)
    qlo, qhi = q * QW, (q + 1) * QW
    a, b = max(qlo, wlo), min(qhi, whi)
    out = []
    if a >= b:
        out.append((qlo, qhi, False))
    else:
        if qlo < a:
            out.append((qlo, a, False))
        out.append((a, b, True))
        if b < qhi:
            out.append((b, qhi, False))
    return out


def _rot(m):
    """Per-m quad processing order, starting at the window's first quad so
    min/eq work lands in the first two of m's iterations, not the last."""
    wq = _window(m)[0] // QW
    return [(wq + i) % NQ for i in range(NQ)]


def _slot(pos):
    return 0 if pos == 0 else (2 if pos % 2 == 1 else 1)


class TileCtx:
    def __init__(self, nc, tile_mod):
        self.nc = nc
        self.tile_mod = tile_mod

    def __enter__(self):
        self.ctx = ExitStack()
        self.ctx.__enter__()
        self.tc = self.tile_mod.TileContext(self.nc)
        self.tc.__enter__()
        return self.tc, self.ctx

    def __exit__(self, *exc):
        self.ctx.__exit__(*exc)
        return self.tc.__exit__(*exc)


def _build_program():
    import concourse.bass as bass
    import concourse.bacc as bacc
    import concourse.tile as tile
    from concourse import mybir

    f16 = mybir.dt.float16
    f32 = mybir.dt.float32
    f8 = mybir.dt.float8e4
    Alu = mybir.AluOpType
    Act = mybir.ActivationFunctionType
    Ax = mybir.AxisListType
    DR = mybir.MatmulPerfMode.DoubleRow

    nc = bacc.Bacc("TRN2", target_bir_lowering=False, debug=False,
                   num_devices=NCORES)

    embT = nc.dram_tensor("embT", [D, N], f8, kind="ExternalInput").ap()
    blkT = nc.dram_tensor("blkT", [128, K_TILES * 1024], f8,
                          kind="ExternalInput").ap()
    eqm = nc.dram_tensor("eqm", [128 * M_TILES, WMAX], f8,
                         kind="ExternalInput").ap()
    out = nc.dram_tensor("out", [128, 1], f32, kind="ExternalOutput").ap()

    with TileCtx(nc, tile) as (tc, ctx):
        persist = ctx.enter_context(tc.tile_pool(name="persist", bufs=1))
        psum = ctx.enter_context(tc.tile_pool(name="ps", bufs=2, space="PSUM"))

        ETq = [persist.tile([128, K_TILES, QW], f8, tag=f"etq{g}",
                            name=f"etq{g}") for g in range(NQ)]
        BlkT = persist.tile([128, K_TILES, 1024], f8, tag="blkt")
        EQM = persist.tile([128, M_TILES * WMAX], f8, tag="eqm")
        P16 = persist.tile([128, 3 * M_TILES, QW], f16, tag="p16")
        minp = persist.tile([128, 2, M_TILES], f32, tag="minp")
        maxF = persist.tile([128, M_TILES], f32, tag="maxF")
        minF = persist.tile([128, M_TILES], f32, tag="minF")
        diffs = persist.tile([128, M_TILES], f32, tag="diffs")
        relu_d = persist.tile([128, M_TILES], f32, tag="relud")
        row_loss = persist.tile([128, 1], f32, tag="rowloss")
        negm = persist.tile([128, 1], f32, tag="negm")

        nc.vector.memset(minp[:], POSF)
        nc.vector.memset(negm[:], -MARGIN_C)

        # PE p-state warmup: ~3us of dummy DR matmuls on garbage SBUF (ETq3
        # is DMA'd last, so reading it now costs nothing); the PE reaches
        # full clock before the first real matmul group.
        wps = psum.tile([128, QW], f32, tag="ps")
        for i in range(16):
            nc.tensor.matmul(wps[:, (i % 4) * 512:(i % 4) * 512 + 512],
                             lhsT=ETq[3][:, 0:2, 0:128],
                             rhs=ETq[3][:, 0:2, 0:512],
                             start=True, stop=True, perf_mode=DR)
        nc.vector.tensor_reduce(out=negm[:], in_=wps[:, 0:8], axis=Ax.X,
                                op=Alu.max)
        nc.vector.memset(negm[:], -MARGIN_C)

        # ---------- loads: 3 parallel DGE queues (sync/scalar/gpsimd) ----
        def dma_etq(eng, g, k):
            eng.dma_start(
                out=ETq[g][:, k, :],
                in_=bass.AP(embT.tensor,
                            embT.offset + k * 128 * N + g * QW,
                            [[N, 128], [1, QW]]))

        # eqm is tiny (0.5MB) — park it all on the slow sync queue, which
        # must stay clear for the final out DMA; ETq/BlkT go on the fast
        # scalar/gpsimd DGE queues as few big dispatches (a queue ring holds
        # ~512 descriptors; more dispatches than that block the engine).
        def dma_quad(eng, g):
            eng.dma_start(
                out=ETq[g][:],
                in_=bass.AP(embT.tensor, embT.offset + g * QW,
                            [[N, 128], [N * 128, K_TILES], [1, QW]]))

        nc.gpsimd.dma_start(out=BlkT[:], in_=blkT)
        dma_quad(nc.scalar, 0)
        dma_quad(nc.gpsimd, 1)
        nc.gpsimd.dma_start(
            out=EQM[:],
            in_=bass.AP(eqm.tensor, eqm.offset,
                        [[WMAX, 128], [WMAX * 128, M_TILES], [1, WMAX]]))
        dma_quad(nc.gpsimd, 2)
        dma_quad(nc.scalar, 3)

        # ---------- mining: m outer, q inner, DVE lagged one iteration ----
        wcnt = [0] * M_TILES

        def lagged(m, q, pos):
            """DVE work for (m,q), emitted one iteration later."""
            sl = _slot(pos)
            slot = P16[:, 3 * m + sl, :]
            qlo = q * QW
            wlo, whi = _window(m)
            for (lo, hi, isw) in _pieces(q, m):
                if not isw:
                    continue
                wc = wcnt[m]
                wcnt[m] += 1
                nc.vector.tensor_reduce(
                    out=minp[:, wc, m:m + 1],
                    in_=slot[:, lo - qlo:hi - qlo],
                    axis=Ax.X, op=Alu.min)
            if pos > 0:
                s0 = P16[:, 3 * m, :]
                if pos == NQ - 1:
                    # s0 was self-folded to [0:1024) during the iteration;
                    # fold the last slot's halves into that
                    nc.vector.tensor_tensor(out=s0[:, 0:1024],
                                            in0=s0[:, 0:1024],
                                            in1=slot[:, 0:1024], op=Alu.max)
                    nc.vector.tensor_tensor(out=s0[:, 0:1024],
                                            in0=s0[:, 0:1024],
                                            in1=slot[:, 1024:2048],
                                            op=Alu.max)
                else:
                    nc.vector.tensor_tensor(out=s0[:], in0=s0[:],
                                            in1=slot[:], op=Alu.max)

        def finale(m):
            s0 = P16[:, 3 * m, :]
            nc.vector.tensor_tensor(out=s0[:, 0:512], in0=s0[:, 0:512],
                                    in1=s0[:, 512:1024], op=Alu.max)
            nc.vector.tensor_reduce(
                out=maxF[:, m:m + 1], in_=s0[:, 0:512],
                axis=Ax.X, op=Alu.max)

        deferred = []

        def flush(now):
            keep = []
            for due, fn in deferred:
                if due <= now:
                    fn()
                else:
                    keep.append((due, fn))
            deferred[:] = keep

        order = [(_rot(w - p)[p], w - p, p) for w in range(NQ + M_TILES - 1)
                 for p in range(NQ) if 0 <= w - p < M_TILES]
        if True:
            for it, (q, m, pos) in enumerate(order):
                qlo = q * QW
                wlo, whi = _window(m)
                ps = psum.tile([128, QW], f32, tag="ps")
                for j in range(2):
                    lhsT = BlkT[:, 2 * j:2 * j + 2, m * 128:(m + 1) * 128]
                    rhs_t = ETq[q][:, 2 * j:2 * j + 2, :]
                    for c in range(4):
                        nc.tensor.matmul(
                            ps[:, c * 512:(c + 1) * 512],
                            lhsT=lhsT,
                            rhs=rhs_t[:, :, c * 512:(c + 1) * 512],
                            start=(j == 0), stop=(j == 1), perf_mode=DR)

                slot = P16[:, 3 * m + _slot(pos), :]
                nc.scalar.copy(slot, ps[:])
                for (lo, hi, isw) in _pieces(q, m):
                    if not isw:
                        continue
                    twb = m * WMAX
                    nc.gpsimd.tensor_tensor(
                        out=slot[:, lo - qlo:hi - qlo],
                        in0=slot[:, lo - qlo:hi - qlo],
                        in1=EQM[:, twb + lo - wlo:twb + hi - wlo],
                        op=Alu.add)
                if pos == NQ - 1:
                    # pre-shrink s0 while this iteration's copy streams, so
                    # the post-copy chain for the last m is shorter
                    s0p = P16[:, 3 * m, :]
                    nc.vector.tensor_tensor(out=s0p[:, 0:1024],
                                            in0=s0p[:, 0:1024],
                                            in1=s0p[:, 1024:2048],
                                            op=Alu.max)
                flush(it)
                has_win = any(w[2] for w in _pieces(q, m))
                due = it + 2 if has_win else it + 1
                deferred.append((due, (lambda mm=m, qq=q, pp=pos:
                                       lagged(mm, qq, pp))))
                if pos == NQ - 1:
                    deferred.append((it + 2, (lambda mm=m: finale(mm))))
        flush(10 ** 9)

        # ---------- finale ----------
        nc.vector.tensor_tensor(out=minF[:], in0=minp[:, 0, :],
                                in1=minp[:, 1, :], op=Alu.min)
        nc.vector.tensor_tensor(out=diffs[:], in0=maxF[:], in1=minF[:],
                                op=Alu.subtract)
        nc.scalar.activation(relu_d[:], diffs[:], Act.Relu, bias=negm[:],
                             accum_out=row_loss[:])
        nc.sync.dma_start(out=out, in_=row_loss[:])

    nc.compile()
    return nc


def _prep_inputs(embeddings, labels):
    import ml_dtypes
    E = np.ascontiguousarray(np.asarray(embeddings, dtype=np.float32))
    lab = np.asarray(labels).reshape(-1)
    assert E.shape == (N, D)

    order = np.argsort(lab, kind="stable")
    E_s = E[order]
    lab_s = lab[order].astype(np.int64)
    assert np.bincount(lab_s).max() <= 129, "label multiplicity > 129"

    En = E_s * (GSCALE / np.linalg.norm(E_s, axis=1, keepdims=True))
    E8 = En.astype(ml_dtypes.float8_e4m3)

    # label -> column range in the sorted order
    starts = np.searchsorted(lab_s, np.arange(lab_s.max() + 2))
    tiles8 = E8.reshape(64, 128, D)
    labt = lab_s.reshape(64, 128)
    in_maps = []
    for c in range(NCORES):
        gsel = [8 * m + c for m in range(M_TILES)]
        blk8 = np.ascontiguousarray(tiles8[gsel].reshape(128 * M_TILES, D))
        blkT_c = np.ascontiguousarray(
            blk8.reshape(1024, K_TILES, 128).transpose(2, 1, 0)
            .reshape(128, K_TILES * 1024))
        lab_blk = labt[gsel].reshape(M_TILES, 128)

        # per-core column permutation: tile m's same-label columns (its
        # "union") go to region [256m, 256m+256), padded with leftovers
        used = np.zeros(N, bool)
        unions = []
        for m in range(M_TILES):
            labs = np.unique(lab_blk[m])
            cols = np.concatenate([np.arange(starts[l], starts[l + 1])
                                   for l in labs])
            assert len(cols) <= 256, f"union too wide: {len(cols)}"
            unions.append(cols)
            used[cols] = True
        leftovers = np.nonzero(~used)[0]
        perm = np.empty(N, np.int64)
        li = 0
        for m in range(M_TILES):
            u = unions[m]
            npad = 256 - len(u)
            perm[256 * m:256 * m + len(u)] = u
            perm[256 * m + len(u):256 * (m + 1)] = leftovers[li:li + npad]
            li += npad
        perm[256 * M_TILES:] = leftovers[li:]
        lab_p = lab_s[perm]
        embT_c = np.ascontiguousarray(E8[perm].T)

        eqm_c = np.zeros((128 * M_TILES, WMAX), np.float32)
        for m in range(M_TILES):
            wlo, whi = _window(m)
            eqm_c[m * 128:(m + 1) * 128, :whi - wlo] = -EQV * (
                lab_p[None, wlo:whi] == lab_blk[m][:, None])
        eqm_c = eqm_c.astype(ml_dtypes.float8_e4m3)
        in_maps.append({
            "embT": embT_c,
            "blkT": blkT_c,
            "eqm": eqm_c,
        })
    return in_maps


def kernel(embeddings, labels):
    from concourse.bass_utils import run_bass_kernel_spmd

    in_maps = _prep_inputs(embeddings, labels)
    nc = _build_program()
    res = run_bass_kernel_spmd(nc, in_maps, core_ids=list(range(NCORES)))
    global LAST_RESULTS
    LAST_RESULTS = res
    total = sum(float(r["out"].sum()) for r in res.results)
    return np.float32(total / (64.0 * N))


LAST_RESULTS = None
